# revision 22
# baseline (speedup 1.0000x reference)
"""Trainium2 Bass kernel for BaselineProtonet (retrieval_knn).

logits[q, c] = -||query_q - proto_c||_2
  proto_c = mean of 64 support embeddings of class c
  embeddings_stacked: [64 classes * (64 support + 64 query), 1024] f32

Sharding (8 cores): 2D-balanced grid, 4 query-groups x 2 class-halves.
Core (a, b) owns query rows 1024a..1024(a+1) and classes 32b..32b+32, so
it reads 1MB of queries + 2MB of support (both fp8 on the wire) instead
of the 4.5MB a pure query shard would need -- the input DMA is the
dominant cost and this is the byte-optimal integer grid. No cross-core
collective (a ncfw collective costs ~65us of control latency in this
runtime, measured).

Host-side shard prep (layout/encoding only, no arithmetic): support is
stored SLOT-major (shard row r holds support vector (class r%32, slot
r//32)) so the one-hot stationary is the same for every proto matmul,
and d-QUARTER-major: the support streams in as four 512KB d-slices,
each feeding an independent protos -> evac -> transpose -> W -> Gram
chain that completes while later quarters are still on the wire; only
the last quarter's ~1.5us chain sits in the tail. Queries are
feature-major fp8 (the kernel uses the rounded values consistently in
the Gram and ||q||^2 terms, so fp8 queries shift each distance rather
than decorrelating the terms).

Per core:
  protos   : per d-quarter, 8 fp8 DoubleRow one-hot matmuls (256 support
             rows each, shared stationary, N=256) -> own PSUM tile
  P^T      : per d-quarter ACT evac (1/64 -> bf16), 2 PE transposes,
             ACT scale -2 -> W fp8 [128, 8, 32]
  Gram     : per d-quarter, 2 fp8 DoubleRow matmuls (lhsT=W pair,
             rhs=Q^T pair, N=512); quarter 0 OPENS the s_ps group
  ||q||^2  : fp8 squares split DVE(0-3,7)/ACT(4,5,6); 8 all-ones fp8
             DoubleRow matmuls broadcast-sum them into s_ps and CLOSE
             each n-half (squares run ~1.2us per chunk on DVE, so they
             get the whole support phase to finish)
  ||p||^2  : per-quarter ACT square-accumulate on the bf16 protos,
             summed on DVE -> [32,1] f32 sqrt bias
  logits   : -sqrt(dist^2) via ACT sqrt(+bias) and DVE negate, four
             n-quarters, each stored by its own DMA as it finishes;
             output [32, 1024] class-major, host transposes into place.
All input DMAs ride the sync ring as seven big starts (few big
single-ring starts avoid cross-ring DMA-semaphore-lane false waits and
sustain ~390 GB/s); a small first query slice gives the PE real work
(the first ||q||^2 matmuls) before the first support quarter lands.
The PE is pre-warmed with ~4.5us of dummy matmuls (HAM clock gate;
otherwise the mid-kernel matmuls run at 1.2 GHz instead of 2.4) and the
ACT tables (Copy/Square/Sqrt) are preloaded off the critical path.
"""

import numpy as np

C = 64          # classes
S = 64          # support per class (== queries per class)
D = 1024        # embedding dim
NCORES = 8
QA = 4          # query groups
CB = 2          # class halves
CL = C // CB    # 32 classes per core
NQ = (C * S) // QA          # 1024 query rows per core
DCH = D // 128              # 8 d-chunks
SCH = (CL * S) // 128       # 16 support row chunks per core
JP = SCH // 2               # 8 DoubleRow chunk pairs

_CACHE = {}


def _emit(nc, tc, sup, qt, out):
    """Emit the per-core tile program.

    sup:   [128, 4*SCH*256] fp8 DRAM  (support, slot-major rows,
                                       d-quarter-major: quarter qd,
                                       chunk j, d-slice; row p of chunk
                                       j = shard row j*128+p, class p%32)
    qt:    [128, DCH*NQ] fp8 DRAM     (queries, swizzled feature-major)
    out:   [CL, NQ] f32 DRAM          (negated distances, class-major)
    """
    from concourse import masks, mybir

    f32 = mybir.dt.float32
    bf16 = mybir.dt.bfloat16
    fp8 = mybir.dt.float8e4
    AF = mybir.ActivationFunctionType
    DR = mybir.MatmulPerfMode.DoubleRow

    with (
        tc.tile_pool(name="sb", bufs=1) as sb,
        tc.tile_pool(name="ps", bufs=1, space="PSUM") as ps,
    ):
        # warm the PE clock first-thing (HAM gate); the dummy matmuls
        # land in the pt_ps bank, which the transposes overwrite later
        wm_in = sb.tile([128, 128], bf16)
        nc.vector.memset(wm_in[:], 0.0)
        pt_ps = ps.tile([128, DCH, CL], bf16)
        wm_ps = ps.tile([128, 128], f32)
        for _ in range(36):
            nc.tensor.matmul(
                wm_ps[:], wm_in[:], wm_in[:], start=True, stop=True
            )

        # ---------------- input DMAs, ALL on the sync ring, 512KB each
        q8 = sb.tile([128, DCH, NQ], fp8)
        s8 = sb.tile([128, 4, SCH, 256], fp8)

        def q_dma(lo, hi, ring=nc.sync):
            ring.dma_start(
                q8[:, lo:hi],
                qt[:, lo * NQ : hi * NQ].rearrange(
                    "p (k q) -> p k q", k=hi - lo
                ),
            )

        def s_dma(qd):
            nc.sync.dma_start(
                s8[:, qd],
                sup[:, qd * SCH * 256 : (qd + 1) * SCH * 256].rearrange(
                    "p (c d) -> p c d", c=SCH
                ),
            )

        q_dma(0, 2)
        s_dma(0)
        q_dma(2, 6)
        q_dma(6, 8)
        s_dma(1)
        s_dma(2)
        s_dma(3)

        # ---------------- constants -------------------------------------
        ident = sb.tile([128, 128], bf16)
        masks.make_identity(nc, ident[:])
        ones = sb.tile([128, 2, CL], fp8)
        nc.gpsimd.memset(ones[:], 1.0)
        # one-hot built on device: oh[p, o, c] = 1 iff c == p % 32,
        # i.e. p - c - 32k == 0 for some k (four diagonal stripe fills)
        oh = sb.tile([128, 2, CL], fp8)
        nc.gpsimd.memset(oh[:], 0.0)
        for k4 in range(4):
            nc.gpsimd.affine_select(
                out=oh[:],
                in_=oh[:],
                compare_op=mybir.AluOpType.not_equal,
                fill=1.0,
                base=-CL * k4,
                pattern=[[0, 2], [-1, CL]],
                channel_multiplier=1,
            )

        # preload the ACT tables off the critical path
        warm_t = sb.tile([1, 1], f32)
        warm_d = sb.tile([1, 1], bf16)
        warm_a = sb.tile([1, 1], f32)
        nc.gpsimd.memset(warm_t[:], 1.0)
        nc.scalar.mul(warm_d[:], warm_t[:], 1.0)
        nc.scalar.activation(warm_d[:], warm_t[:], AF.Square, accum_out=warm_a[:])
        nc.scalar.activation(warm_t[:], warm_t[:], AF.Sqrt)

        # ---------------- prototypes, all quarters, highest PE priority
        s8v = s8[:].rearrange("p qd (jp o) d -> p qd jp o d", o=2)
        p_q = [ps.tile([CL, 256], f32, name=f"p_q{qd}") for qd in range(4)]
        for qd in range(4):
            for jp in range(JP):
                nc.tensor.matmul(
                    p_q[qd][:],
                    oh[:],
                    s8v[:, qd, jp],
                    start=(jp == 0),
                    stop=(jp == JP - 1),
                    perf_mode=DR,
                )

        # ---------------- ||q||^2 squares (fp8 out): ACT 4,5 early,
        # DVE 0-3; the late chunks (6: ACT, 7: DVE) are emitted below
        qsq = sb.tile([128, DCH, NQ], fp8)
        for k in (4, 5):
            nc.scalar.activation(qsq[:, k], q8[:, k], AF.Square)
        for k in (0, 1, 2, 3):
            nc.vector.tensor_mul(qsq[:, k], q8[:, k], q8[:, k])

        # ||q||^2 ones-matmuls OPEN the s_ps group (fp8 DoubleRow over
        # chunk pairs): ready mid-stream, they keep the PE warm while
        # the support quarters are still on the wire; emitted in chunk-
        # arrival order (pairs 0,1 early; 2 with sq4/5; 3 last)
        s_ps = ps.tile([CL, NQ], f32)
        qsqv = qsq[:].rearrange("p (m o) q -> p m o q", o=2)

        def qsq_mm(m, start):
            for n in range(2):
                nc.tensor.matmul(
                    s_ps[:, 512 * n : 512 * (n + 1)],
                    ones[:],
                    qsqv[:, m, :, 512 * n : 512 * (n + 1)],
                    start=start,
                    stop=False,
                    perf_mode=DR,
                )

        for m in (0, 1, 2):
            qsq_mm(m, m == 0)

        # ---------------- per-quarter evac / transpose / W / Gram chain
        q8v = q8[:].rearrange("p (kp o) q -> p kp o q", o=2)
        psb = [sb.tile([CL, 256], bf16, name=f"psb{qd}") for qd in range(4)]
        pn_dump = sb.tile([CL, D], bf16)
        pn_q = [sb.tile([CL, 1], f32, name=f"pn_q{qd}") for qd in range(4)]
        W = sb.tile([128, DCH, CL], fp8)
        for qd in range(4):
            nc.scalar.mul(psb[qd][:], p_q[qd][:], 1.0 / S)
            for o in range(2):
                nc.tensor.transpose(
                    pt_ps[:, 2 * qd + o],
                    psb[qd][:, 128 * o : 128 * (o + 1)],
                    ident[0:CL, 0:CL],
                )
            nc.scalar.mul(
                W[:, 2 * qd : 2 * qd + 2], pt_ps[:, 2 * qd : 2 * qd + 2], -2.0
            )
            for n in range(2):
                nc.tensor.matmul(
                    s_ps[:, 512 * n : 512 * (n + 1)],
                    W[:, 2 * qd : 2 * qd + 2],
                    q8v[:, qd, :, 512 * n : 512 * (n + 1)],
                    start=False,
                    stop=(qd == 3),
                    perf_mode=DR,
                )
            nc.scalar.activation(
                pn_dump[:, 256 * qd : 256 * (qd + 1)],
                psb[qd][:],
                AF.Square,
                accum_out=pn_q[qd][:],
            )
            if qd == 1:
                # late square chunks, emitted mid-chain so the engine
                # queues reach them right as query chunks 6,7 land
                nc.scalar.activation(qsq[:, 6], q8[:, 6], AF.Square)
                nc.vector.tensor_mul(qsq[:, 7], q8[:, 7], q8[:, 7])
            if qd == 2:
                qsq_mm(3, False)

        # ||p||^2 quarters summed on DVE
        pn_ab = sb.tile([CL, 2], f32)
        pn_col = sb.tile([CL, 1], f32)
        nc.vector.tensor_add(pn_ab[:, 0:1], pn_q[0][:], pn_q[1][:])
        nc.vector.tensor_add(pn_ab[:, 1:2], pn_q[2][:], pn_q[3][:])
        nc.vector.tensor_add(pn_col[:], pn_ab[:, 0:1], pn_ab[:, 1:2])

        # ------- sqrt(+||p||^2), negate, store (quarters pipelined) -----
        lt = sb.tile([CL, NQ], f32)
        for qi in range(4):
            s = slice(256 * qi, 256 * (qi + 1))
            nc.scalar.activation(
                lt[:, s], s_ps[:, s], AF.Sqrt, bias=pn_col[:, 0:1]
            )
            nc.vector.tensor_scalar_mul(lt[:, s], lt[:, s], -1.0)
            nc.sync.dma_start(out[:, s], lt[:, s])


def _build():
    if "nc" in _CACHE:
        return _CACHE["nc"]
    from concourse import bacc, mybir, tile

    f32 = mybir.dt.float32
    fp8 = mybir.dt.float8e4
    nc = bacc.Bacc(
        "TRN2",
        target_bir_lowering=False,
        debug=False,
        enable_asserts=False,
        num_devices=NCORES,
    )
    sup = nc.dram_tensor("sup", [128, 4 * SCH * 256], fp8, kind="ExternalInput").ap()
    qt = nc.dram_tensor("qt", [128, DCH * NQ], fp8, kind="ExternalInput").ap()
    out = nc.dram_tensor("out", [CL, NQ], f32, kind="ExternalOutput").ap()
    with tile.TileContext(nc) as tc:
        _emit(nc, tc, sup, qt, out)
    nc.compile()
    _CACHE["nc"] = nc
    return nc


def _shard(embeddings):
    import ml_dtypes

    emb = np.asarray(embeddings, dtype=np.float32).reshape(C, 2 * S, D)
    # support halves: classes 32b..32b+32, SLOT-major shard rows
    # (r = s*32 + c_local), swizzled [128, 4, SCH, 256] d-quarter-major
    # (row p of chunk j = shard row j*128+p), fp8
    sups = []
    for b in range(CB):
        shard = (
            emb[CL * b : CL * (b + 1), :S, :]
            .transpose(1, 0, 2)
            .reshape(CL * S, D)
        )
        sw = shard.reshape(SCH, 128, 4, 256).transpose(1, 2, 0, 3)
        sups.append(
            np.ascontiguousarray(
                sw.astype(ml_dtypes.float8_e4m3).reshape(128, 4 * SCH * 256)
            )
        )
    # query groups: rows 1024a..1024(a+1) of the query set, feature-major
    qry = emb[:, S:, :].reshape(C * S, D)
    qts = []
    for a in range(QA):
        q = qry[NQ * a : NQ * (a + 1)]
        qt_i = q.T.reshape(DCH, 128, NQ).transpose(1, 0, 2)
        qts.append(
            np.ascontiguousarray(
                qt_i.astype(ml_dtypes.float8_e4m3).reshape(128, DCH * NQ)
            )
        )
    in_maps = []
    for i in range(NCORES):
        a, b = i // CB, i % CB
        in_maps.append({"sup": sups[b], "qt": qts[a]})
    return in_maps


def _assemble(outs):
    """outs: per-core [CL, NQ] blocks -> full [C*S, C] logits."""
    logits = np.empty((C * S, C), dtype=np.float32)
    for i in range(NCORES):
        a, b = i // CB, i % CB
        logits[NQ * a : NQ * (a + 1), CL * b : CL * (b + 1)] = outs[i].T
    return logits


def kernel(embeddings_stacked, n_classes, n_support, **_unused):
    assert int(n_classes) == C and int(n_support) == S
    emb = np.asarray(embeddings_stacked)
    assert emb.shape == (C * 2 * S, D), emb.shape

    from concourse import bass_utils

    nc = _build()
    in_maps = _shard(emb)
    try:
        res = bass_utils.run_bass_kernel_spmd(
            nc, in_maps, core_ids=list(range(NCORES))
        )
    except Exception:
        # transient device/runtime hiccups have been observed; retry once
        res = bass_utils.run_bass_kernel_spmd(
            nc, in_maps, core_ids=list(range(NCORES))
        )
    return _assemble([res.results[i]["out"] for i in range(NCORES)])


if __name__ == "__main__":
    rng = np.random.default_rng(0)
    emb = rng.standard_normal((C * 2 * S, D), dtype=np.float32)
    got = kernel(emb, C, S)
    print("kernel output", got.shape, got.dtype)


# revision 25
# speedup vs baseline: 1.0225x; 1.0225x over previous
"""Trainium2 Bass kernel for BaselineProtonet (retrieval_knn).

logits[q, c] = -||query_q - proto_c||_2
  proto_c = mean of 64 support embeddings of class c
  embeddings_stacked: [64 classes * (64 support + 64 query), 1024] f32

Sharding (8 cores): 2D-balanced grid, 4 query-groups x 2 class-halves.
Core (a, b) owns query rows 1024a..1024(a+1) and classes 32b..32b+32, so
it reads 1MB of queries + 2MB of support (both fp8 on the wire) instead
of the 4.5MB a pure query shard would need -- the input DMA is the
dominant cost and this is the byte-optimal integer grid. No cross-core
collective (a ncfw collective costs ~65us of control latency in this
runtime, measured).

Host-side shard prep (layout/encoding only, no arithmetic): support is
stored SLOT-major (shard row r holds support vector (class r%32, slot
r//32)) so the one-hot stationary is the same for every proto matmul,
and d-QUARTER-major: the support streams in as four 512KB d-slices,
each feeding an independent protos -> evac -> transpose -> W -> Gram
chain that completes while later quarters are still on the wire; only
the last quarter's ~1.5us chain sits in the tail. Queries are
feature-major fp8 (the kernel uses the rounded values consistently in
the Gram and ||q||^2 terms, so fp8 queries shift each distance rather
than decorrelating the terms).

Per core:
  protos   : per d-quarter, 8 fp8 DoubleRow one-hot matmuls (256 support
             rows each, shared stationary, N=256) -> own PSUM tile
  P^T      : per d-quarter DVE evac (1/64 -> bf16), 2 PE transposes,
             DVE scale -2 -> W fp8 [128, 8, 32] (DVE, not ACT, so the
             per-quarter chain never queues behind the ACT squares)
  Gram     : per d-quarter, 2 fp8 DoubleRow matmuls (lhsT=W pair,
             rhs=Q^T pair, N=512); quarter 0 OPENS the s_ps group
  ||q||^2  : fp8 squares split DVE(0,1,7)/ACT(2-6); 8 all-ones fp8
             DoubleRow matmuls broadcast-sum them into s_ps (squares
             run ~1.2us per chunk, so they are spread over both
             engines and the whole support phase)
  ||p||^2  : per-quarter DVE square + free-dim reduce on the bf16
             protos, summed -> [32,1] f32 sqrt bias
  logits   : -sqrt(dist^2) via ACT sqrt(+bias) and DVE negate, four
             n-quarters, each stored by its own DMA as it finishes;
             output [32, 1024] class-major, host transposes into place.
All input DMAs ride the sync ring as seven big starts (few big
single-ring starts avoid cross-ring DMA-semaphore-lane false waits and
sustain ~390 GB/s); a small first query slice gives the PE real work
(the first ||q||^2 matmuls) before the first support quarter lands.
The PE is pre-warmed with ~4.5us of dummy matmuls (HAM clock gate;
otherwise the mid-kernel matmuls run at 1.2 GHz instead of 2.4) and the
ACT tables (Copy/Square/Sqrt) are preloaded off the critical path.
"""

import numpy as np

C = 64          # classes
S = 64          # support per class (== queries per class)
D = 1024        # embedding dim
NCORES = 8
QA = 4          # query groups
CB = 2          # class halves
CL = C // CB    # 32 classes per core
NQ = (C * S) // QA          # 1024 query rows per core
DCH = D // 128              # 8 d-chunks
SCH = (CL * S) // 128       # 16 support row chunks per core
JP = SCH // 2               # 8 DoubleRow chunk pairs

_CACHE = {}


def _emit(nc, tc, sup, qt, out):
    """Emit the per-core tile program.

    sup:   [128, 4*SCH*256] fp8 DRAM  (support, slot-major rows,
                                       d-quarter-major: quarter qd,
                                       chunk j, d-slice; row p of chunk
                                       j = shard row j*128+p, class p%32)
    qt:    [128, DCH*NQ] fp8 DRAM     (queries, swizzled feature-major)
    out:   [CL, NQ] f32 DRAM          (negated distances, class-major)
    """
    from concourse import masks, mybir

    f32 = mybir.dt.float32
    bf16 = mybir.dt.bfloat16
    fp8 = mybir.dt.float8e4
    AF = mybir.ActivationFunctionType
    DR = mybir.MatmulPerfMode.DoubleRow

    with (
        tc.tile_pool(name="sb", bufs=1) as sb,
        tc.tile_pool(name="ps", bufs=1, space="PSUM") as ps,
    ):
        # warm the PE clock first-thing (HAM gate); the dummy matmuls
        # land in the pt_ps bank, which the transposes overwrite later
        wm_in = sb.tile([128, 128], bf16)
        nc.vector.memset(wm_in[:], 0.0)
        pt_ps = ps.tile([128, DCH, CL], bf16)
        wm_ps = ps.tile([128, 128], f32)
        for _ in range(36):
            nc.tensor.matmul(
                wm_ps[:], wm_in[:], wm_in[:], start=True, stop=True
            )

        # ---------------- input DMAs, ALL on the sync ring, 512KB each
        q8 = sb.tile([128, DCH, NQ], fp8)
        s8 = sb.tile([128, 4, SCH, 256], fp8)

        def q_dma(lo, hi, ring=nc.sync):
            ring.dma_start(
                q8[:, lo:hi],
                qt[:, lo * NQ : hi * NQ].rearrange(
                    "p (k q) -> p k q", k=hi - lo
                ),
            )

        def s_dma(qd):
            nc.sync.dma_start(
                s8[:, qd],
                sup[:, qd * SCH * 256 : (qd + 1) * SCH * 256].rearrange(
                    "p (c d) -> p c d", c=SCH
                ),
            )

        q_dma(0, 2)
        s_dma(0)
        q_dma(2, 6)
        q_dma(6, 8)
        s_dma(1)
        s_dma(2)
        s_dma(3)

        # ---------------- constants -------------------------------------
        ident = sb.tile([128, 128], bf16)
        masks.make_identity(nc, ident[:])
        ones = sb.tile([128, 2, CL], fp8)
        nc.gpsimd.memset(ones[:], 1.0)
        # one-hot built on device: oh[p, o, c] = 1 iff c == p % 32,
        # i.e. p - c - 32k == 0 for some k (four diagonal stripe fills)
        oh = sb.tile([128, 2, CL], fp8)
        nc.gpsimd.memset(oh[:], 0.0)
        for k4 in range(4):
            nc.gpsimd.affine_select(
                out=oh[:],
                in_=oh[:],
                compare_op=mybir.AluOpType.not_equal,
                fill=1.0,
                base=-CL * k4,
                pattern=[[0, 2], [-1, CL]],
                channel_multiplier=1,
            )

        # preload the ACT tables off the critical path
        warm_t = sb.tile([1, 1], f32)
        warm_d = sb.tile([1, 1], bf16)
        warm_a = sb.tile([1, 1], f32)
        nc.gpsimd.memset(warm_t[:], 1.0)
        nc.scalar.mul(warm_d[:], warm_t[:], 1.0)
        nc.scalar.activation(warm_d[:], warm_t[:], AF.Square, accum_out=warm_a[:])
        nc.scalar.activation(warm_t[:], warm_t[:], AF.Sqrt)

        # ---------------- prototypes, all quarters, highest PE priority
        s8v = s8[:].rearrange("p qd (jp o) d -> p qd jp o d", o=2)
        p_q = [ps.tile([CL, 256], f32, name=f"p_q{qd}") for qd in range(4)]
        for qd in range(4):
            for jp in range(JP):
                nc.tensor.matmul(
                    p_q[qd][:],
                    oh[:],
                    s8v[:, qd, jp],
                    start=(jp == 0),
                    stop=(jp == JP - 1),
                    perf_mode=DR,
                )

        # ---------------- ||q||^2 squares (fp8 out): ACT 4,5 early,
        # DVE 0-3; the late chunks (6: ACT, 7: DVE) are emitted below
        qsq = sb.tile([128, DCH, NQ], fp8)
        for k in (0, 1):
            nc.vector.tensor_mul(qsq[:, k], q8[:, k], q8[:, k])
        for k in (2, 3, 4, 5):
            nc.scalar.activation(qsq[:, k], q8[:, k], AF.Square)

        # ||q||^2 ones-matmuls OPEN the s_ps group (fp8 DoubleRow over
        # chunk pairs): ready mid-stream, they keep the PE warm while
        # the support quarters are still on the wire; emitted in chunk-
        # arrival order (pairs 0,1 early; 2 with sq4/5; 3 last)
        s_ps = ps.tile([CL, NQ], f32)
        qsqv = qsq[:].rearrange("p (m o) q -> p m o q", o=2)

        def qsq_mm(m, start):
            for n in range(2):
                nc.tensor.matmul(
                    s_ps[:, 512 * n : 512 * (n + 1)],
                    ones[:],
                    qsqv[:, m, :, 512 * n : 512 * (n + 1)],
                    start=start,
                    stop=False,
                    perf_mode=DR,
                )

        for m in (0, 1, 2):
            qsq_mm(m, m == 0)

        # ---------------- per-quarter evac / transpose / W / Gram chain
        q8v = q8[:].rearrange("p (kp o) q -> p kp o q", o=2)
        psb = [sb.tile([CL, 256], bf16, name=f"psb{qd}") for qd in range(4)]
        pn_dump = sb.tile([CL, D], f32)
        pn_q = [sb.tile([CL, 1], f32, name=f"pn_q{qd}") for qd in range(4)]
        W = sb.tile([128, DCH, CL], fp8)
        for qd in range(4):
            nc.vector.tensor_scalar_mul(psb[qd][:], p_q[qd][:], 1.0 / S)
            for o in range(2):
                nc.tensor.transpose(
                    pt_ps[:, 2 * qd + o],
                    psb[qd][:, 128 * o : 128 * (o + 1)],
                    ident[0:CL, 0:CL],
                )
            nc.vector.tensor_scalar_mul(
                W[:, 2 * qd : 2 * qd + 2], pt_ps[:, 2 * qd : 2 * qd + 2], -2.0
            )
            for n in range(2):
                nc.tensor.matmul(
                    s_ps[:, 512 * n : 512 * (n + 1)],
                    W[:, 2 * qd : 2 * qd + 2],
                    q8v[:, qd, :, 512 * n : 512 * (n + 1)],
                    start=False,
                    stop=(qd == 3),
                    perf_mode=DR,
                )
            nc.vector.tensor_mul(
                pn_dump[:, 256 * qd : 256 * (qd + 1)], psb[qd][:], psb[qd][:]
            )
            nc.vector.tensor_reduce(
                pn_q[qd][:],
                pn_dump[:, 256 * qd : 256 * (qd + 1)],
                axis=mybir.AxisListType.X,
                op=mybir.AluOpType.add,
            )
            if qd == 1:
                # late square chunks, emitted mid-chain so the engine
                # queues reach them right as query chunks 6,7 land
                nc.scalar.activation(qsq[:, 6], q8[:, 6], AF.Square)
                nc.vector.tensor_mul(qsq[:, 7], q8[:, 7], q8[:, 7])
            if qd == 2:
                qsq_mm(3, False)

        # ||p||^2 quarters summed on DVE
        pn_ab = sb.tile([CL, 2], f32)
        pn_col = sb.tile([CL, 1], f32)
        nc.vector.tensor_add(pn_ab[:, 0:1], pn_q[0][:], pn_q[1][:])
        nc.vector.tensor_add(pn_ab[:, 1:2], pn_q[2][:], pn_q[3][:])
        nc.vector.tensor_add(pn_col[:], pn_ab[:, 0:1], pn_ab[:, 1:2])

        # ------- sqrt(+||p||^2), negate, store (quarters pipelined) -----
        lt = sb.tile([CL, NQ], f32)
        for qi in range(4):
            s = slice(256 * qi, 256 * (qi + 1))
            nc.scalar.activation(
                lt[:, s], s_ps[:, s], AF.Sqrt, bias=pn_col[:, 0:1]
            )
            nc.vector.tensor_scalar_mul(lt[:, s], lt[:, s], -1.0)
            ring = nc.sync if qi % 2 == 0 else nc.scalar
            ring.dma_start(out[:, s], lt[:, s])


def _build():
    if "nc" in _CACHE:
        return _CACHE["nc"]
    from concourse import bacc, mybir, tile

    f32 = mybir.dt.float32
    fp8 = mybir.dt.float8e4
    nc = bacc.Bacc(
        "TRN2",
        target_bir_lowering=False,
        debug=False,
        enable_asserts=False,
        num_devices=NCORES,
    )
    sup = nc.dram_tensor("sup", [128, 4 * SCH * 256], fp8, kind="ExternalInput").ap()
    qt = nc.dram_tensor("qt", [128, DCH * NQ], fp8, kind="ExternalInput").ap()
    out = nc.dram_tensor("out", [CL, NQ], f32, kind="ExternalOutput").ap()
    with tile.TileContext(nc) as tc:
        _emit(nc, tc, sup, qt, out)
    nc.compile()
    _CACHE["nc"] = nc
    return nc


def _shard(embeddings):
    import ml_dtypes

    emb = np.asarray(embeddings, dtype=np.float32).reshape(C, 2 * S, D)
    # support halves: classes 32b..32b+32, SLOT-major shard rows
    # (r = s*32 + c_local), swizzled [128, 4, SCH, 256] d-quarter-major
    # (row p of chunk j = shard row j*128+p), fp8
    sups = []
    for b in range(CB):
        shard = (
            emb[CL * b : CL * (b + 1), :S, :]
            .transpose(1, 0, 2)
            .reshape(CL * S, D)
        )
        sw = shard.reshape(SCH, 128, 4, 256).transpose(1, 2, 0, 3)
        sups.append(
            np.ascontiguousarray(
                sw.astype(ml_dtypes.float8_e4m3).reshape(128, 4 * SCH * 256)
            )
        )
    # query groups: rows 1024a..1024(a+1) of the query set, feature-major
    qry = emb[:, S:, :].reshape(C * S, D)
    qts = []
    for a in range(QA):
        q = qry[NQ * a : NQ * (a + 1)]
        qt_i = q.T.reshape(DCH, 128, NQ).transpose(1, 0, 2)
        qts.append(
            np.ascontiguousarray(
                qt_i.astype(ml_dtypes.float8_e4m3).reshape(128, DCH * NQ)
            )
        )
    in_maps = []
    for i in range(NCORES):
        a, b = i // CB, i % CB
        in_maps.append({"sup": sups[b], "qt": qts[a]})
    return in_maps


def _assemble(outs):
    """outs: per-core [CL, NQ] blocks -> full [C*S, C] logits."""
    logits = np.empty((C * S, C), dtype=np.float32)
    for i in range(NCORES):
        a, b = i // CB, i % CB
        logits[NQ * a : NQ * (a + 1), CL * b : CL * (b + 1)] = outs[i].T
    return logits


def kernel(embeddings_stacked, n_classes, n_support, **_unused):
    assert int(n_classes) == C and int(n_support) == S
    emb = np.asarray(embeddings_stacked)
    assert emb.shape == (C * 2 * S, D), emb.shape

    from concourse import bass_utils

    nc = _build()
    in_maps = _shard(emb)
    try:
        res = bass_utils.run_bass_kernel_spmd(
            nc, in_maps, core_ids=list(range(NCORES))
        )
    except Exception:
        # transient device/runtime hiccups have been observed; retry once
        res = bass_utils.run_bass_kernel_spmd(
            nc, in_maps, core_ids=list(range(NCORES))
        )
    return _assemble([res.results[i]["out"] for i in range(NCORES)])


if __name__ == "__main__":
    rng = np.random.default_rng(0)
    emb = rng.standard_normal((C * 2 * S, D), dtype=np.float32)
    got = kernel(emb, C, S)
    print("kernel output", got.shape, got.dtype)
